# revision 5
# baseline (speedup 1.0000x reference)
"""Trainium2 Bass kernel for nn_AttnDecoderRNN (batch=1 single-step decode).

Strategy (vocab/tensor parallel, per the sharding hint):
  - out_W (50257x512, dominant memory traffic) is sharded across 8 cores on
    the vocab dim; each core computes its logits shard on the TensorEngine,
    then AllGather + replicated log_softmax tail.
  - The small recurrent chain (attention + comb + 2-layer bi-GRU) is
    replicated on every core; matvecs run on the TensorEngine in row form:
    out[1,N] = sum_b matmul(lhsT=x_col_block[128,1], rhs=W_pm_block[128,N]).
  - Weights are pre-swizzled on the host into partition-major layout
    pm[p, b*N+n] = W[b*128+p, n] so every weight DMA is contiguous per
    partition. Biases are added with DVE ops (out_b in the gathered
    partition-major form after the AllGather).
"""

import numpy as np

import concourse.bass as bass
import concourse.bacc as bacc
import concourse.mybir as mybir
import concourse.tile as tile
from concourse.bass_utils import run_bass_kernel_spmd

F32 = mybir.dt.float32
I32 = mybir.dt.int32
AF = mybir.ActivationFunctionType
ALU = mybir.AluOpType

H = 256
L = 512          # MAXLEN
V = 50257
NC = 8           # cores
VS = 6656        # padded vocab shard per core (13*512, 52*128)
VPAD = VS * NC   # 53248
FW = VPAD // 128  # 416
NG_W = [1536, 1536, 1536, 1536, 512]   # N-groups of the shard (sum = VS)
NEG_BIG = -1.0e4  # bias padding: exp() -> 0, host trims these entries

LAST_EXEC_NS = None
_CACHE = {}


def _pm(m):
    """[K, N] -> [128, (K//128)*N] with pm[p, b*N+n] = m[b*128+p, n]."""
    k, n = m.shape
    assert k % 128 == 0
    return np.ascontiguousarray(
        m.reshape(k // 128, 128, n).transpose(1, 0, 2).reshape(128, (k // 128) * n))


def _build_nc():
    nc = bacc.Bacc("TRN2", target_bir_lowering=False, debug=False,
                   enable_asserts=False, num_devices=NC)

    d = {}
    def inp(name, shape, dt=F32):
        d[name] = nc.dram_tensor(name, shape, dt, kind="ExternalInput").ap()
    inp("idx", [1, 1], I32)
    inp("emb", [V, H])
    inp("attnw_pm", [128, 4 * L])
    inp("cv_pm", [128, 4 * L])
    inp("comb_pm", [128, 6 * H])
    inp("ih0_pm", [128, 2 * 2 * 768])
    inp("hh0_pm", [128, 2 * 2 * 768])
    inp("ih1_pm", [128, 2 * 4 * 768])
    inp("hh1_pm", [128, 2 * 2 * 768])
    inp("attnb", [1, L])
    inp("combb", [1, H])
    inp("brz_sum", [1, 4 * 512])   # (b_ih+b_hh)[0:512] per cell
    inp("bihn", [1, 4 * 256])      # b_ih[512:768] per cell
    inp("bhhn", [1, 4 * 256])      # b_hh[512:768] per cell
    inp("hidden_cols", [128, 8])
    inp("hidden_rows", [1, 4 * H])
    inp("consts", [128, 132])
    inp("outw_pm", [128, 4 * VS])
    inp("outb_pm", [128, FW])      # full padded out_b, partition-major

    d["logp_out"] = nc.dram_tensor("logp_out", [128, FW], F32, kind="ExternalOutput").ap()
    d["nh_out"] = nc.dram_tensor("nh_out", [1, 4 * H], F32, kind="ExternalOutput").ap()
    d["aw_out"] = nc.dram_tensor("aw_out", [1, L], F32, kind="ExternalOutput").ap()

    with tile.TileContext(nc) as tc:
        _emit(nc, tc, d)
    nc.compile()
    return nc


def _emit(nc, tc, d):
    from contextlib import ExitStack
    with ExitStack() as ctx:
        sb = ctx.enter_context(tc.tile_pool(name="sb", bufs=1))
        wpool = ctx.enter_context(tc.tile_pool(name="wpool", bufs=8))
        lgp = ctx.enter_context(tc.tile_pool(name="lgp", bufs=3))
        rows = ctx.enter_context(tc.tile_pool(name="rows", bufs=1))
        mvp = ctx.enter_context(tc.tile_pool(name="mvp", bufs=3, space="PSUM"))
        gp = ctx.enter_context(tc.tile_pool(name="gp", bufs=1, space="PSUM"))
        cp = ctx.enter_context(tc.tile_pool(name="cp", bufs=1, space="PSUM"))
        dram = ctx.enter_context(tc.tile_pool(name="dram", bufs=1, space="DRAM"))

        # ---- constants & index ----
        consts = sb.tile([128, 132], F32, tag="consts")
        nc.sync.dma_start(consts[:], d["consts"][:])
        one = consts[0:1, 0:1]
        ones_col = consts[:, 0:1]
        ones_row = consts[0:1, 4:132]

        idx_sb = sb.tile([1, 1], I32, tag="idx")
        nc.sync.dma_start(idx_sb[:], d["idx"][:])
        idx2 = sb.tile([2, 1], I32, tag="idx2")
        nc.gpsimd.partition_broadcast(idx2[:], idx_sb[:], channels=2)
        emb2 = sb.tile([2, H], F32, tag="emb2")
        nc.gpsimd.indirect_dma_start(
            out=emb2[:], out_offset=None, in_=d["emb"][:],
            in_offset=bass.IndirectOffsetOnAxis(ap=idx2[:, 0:1], axis=0))
        emb_row = emb2[0:1, :]

        # ---- small-weight / bias loads ----
        def load(name, shape):
            t = sb.tile(shape, F32, tag=name)
            nc.sync.dma_start(t[:], d[name][:])
            return t

        attnw_pm = load("attnw_pm", [128, 4 * L])
        cv_pm = load("cv_pm", [128, 4 * L])
        comb_pm = load("comb_pm", [128, 6 * H])
        ih0_pm = load("ih0_pm", [128, 2 * 2 * 768])
        hh0_pm = load("hh0_pm", [128, 2 * 2 * 768])
        ih1_pm = load("ih1_pm", [128, 2 * 4 * 768])
        hh1_pm = load("hh1_pm", [128, 2 * 2 * 768])
        attnb = load("attnb", [1, L])
        combb = load("combb", [1, H])
        brz_sum = load("brz_sum", [1, 4 * 512])
        bihn = load("bihn", [1, 4 * 256])
        bhhn = load("bhhn", [1, 4 * 256])
        hcols = load("hidden_cols", [128, 8])
        hrows = load("hidden_rows", [1, 4 * H])
        outb_pm = load("outb_pm", [128, FW])

        # ---- out_W shard slabs (g-major so slots recycle across groups) ----
        goff = [0]
        for w in NG_W:
            goff.append(goff[-1] + w)
        wslab = {}
        for g, gw in enumerate(NG_W):
            for b in range(4):
                t = wpool.tile([128, gw], F32, tag="wsl")
                nc.sync.dma_start(
                    t[:], d["outw_pm"][:, b * VS + goff[g]: b * VS + goff[g] + gw])
                wslab[(b, g)] = t

        # warm the exp/tanh table set early (overlaps the DMA stream)
        scr1 = rows.tile([1, 1], F32, tag="scr1")
        nc.scalar.activation(scr1[:], one, AF.Exp)

        # ---- helpers ----
        def mv(ps_ap, w_tile, base, n_tot, xs, n0, n1):
            for b, xb in enumerate(xs):
                nc.tensor.matmul(
                    ps_ap, lhsT=xb,
                    rhs=w_tile[:, base + b * n_tot + n0: base + b * n_tot + n1],
                    start=(b == 0), stop=(b == len(xs) - 1))

        def row2col(row_ap, nchunks, dst_tag):
            ps = cp.tile([128, 4], F32, tag="cps")
            for c in range(nchunks):
                nc.tensor.matmul(ps[:, c:c + 1],
                                 lhsT=row_ap[0:1, c * 128:(c + 1) * 128],
                                 rhs=one, start=True, stop=True)
            col = sb.tile([128, nchunks], F32, tag=dst_tag)
            nc.vector.tensor_copy(col[:], ps[:, 0:nchunks])
            return col

        # ---- attention ----
        emb_col = row2col(emb_row, 2, "emb_col")
        xs_attn = [emb_col[:, 0:1], emb_col[:, 1:2], hcols[:, 0:1], hcols[:, 1:2]]
        ps_attn = gp.tile([1, 768], F32, tag="ga")
        mv(ps_attn[0:1, 0:512], attnw_pm, 0, L, xs_attn, 0, L)
        attnlog = rows.tile([1, L], F32, tag="attnlog")
        nc.vector.tensor_tensor(attnlog[:], ps_attn[0:1, 0:512], attnb[:], op=ALU.add)
        expw = rows.tile([1, L], F32, tag="expw")
        ssum = rows.tile([1, 1], F32, tag="ssum")
        nc.scalar.activation(expw[:], attnlog[:], AF.Exp, accum_out=ssum[:])
        rsum = rows.tile([1, 1], F32, tag="rsum")
        nc.vector.reciprocal(rsum[:], ssum[:])
        aw_row = rows.tile([1, L], F32, tag="aw_row")
        nc.vector.tensor_scalar(aw_row[:], expw[:], rsum[:, 0:1], None, op0=ALU.mult)
        nc.sync.dma_start(d["aw_out"][:], aw_row[:])

        aw_col = row2col(aw_row[:], 4, "aw_col")
        ps_app = gp.tile([1, 768], F32, tag="gb")
        mv(ps_app[0:1, 0:512], cv_pm, 0, L,
           [aw_col[:, b:b + 1] for b in range(4)], 0, L)
        app_row = rows.tile([1, L], F32, tag="app_row")
        nc.scalar.copy(app_row[:], ps_app[0:1, 0:512])
        app_col = row2col(app_row[:], 4, "app_col")

        # ---- comb + relu ----
        xs_comb = [emb_col[:, 0:1], emb_col[:, 1:2]] + \
                  [app_col[:, b:b + 1] for b in range(4)]
        ps_x = gp.tile([1, 768], F32, tag="ga")
        mv(ps_x[0:1, 0:H], comb_pm, 0, H, xs_comb, 0, H)
        xb_row = rows.tile([1, H], F32, tag="xb_row")
        nc.vector.tensor_tensor(xb_row[:], ps_x[0:1, 0:H], combb[:], op=ALU.add)
        x_row = rows.tile([1, H], F32, tag="x_row")
        nc.scalar.activation(x_row[:], xb_row[:], AF.Relu)
        x_col = row2col(x_row[:], 2, "x_col")

        # ---- GRU cells ----
        hid_out = rows.tile([1, 4 * H], F32, tag="hid_out")

        def gru_cell(ci, xs, ih_pm, ih_base, h_cols, hh_pm, hh_base, h_row_ap,
                     out_row_ap):
            A = gp.tile([1, 768], F32, tag="ga")
            B = gp.tile([1, 768], F32, tag="gb")
            mv(A[0:1, 0:512], ih_pm, ih_base, 768, xs, 0, 512)
            mv(A[0:1, 512:768], ih_pm, ih_base, 768, xs, 512, 768)
            mv(B[0:1, 0:512], hh_pm, hh_base, 768, h_cols, 0, 512)
            mv(B[0:1, 512:768], hh_pm, hh_base, 768, h_cols, 512, 768)
            trz = rows.tile([1, 512], F32, tag="trz")
            nc.vector.tensor_tensor(trz[:], A[0:1, 0:512],
                                    brz_sum[0:1, ci * 512:(ci + 1) * 512], op=ALU.add)
            nc.vector.tensor_tensor(trz[:], trz[:], B[0:1, 0:512], op=ALU.add)
            sig = rows.tile([1, 512], F32, tag="sig")
            # sigmoid(t) = 0.5*tanh(t/2)+0.5 (stays in the exp/tanh table set)
            nc.scalar.activation(sig[:], trz[:], AF.Tanh, scale=0.5)
            nc.vector.tensor_scalar(sig[:], sig[:], 0.5, 0.5,
                                    op0=ALU.mult, op1=ALU.add)
            bnp = rows.tile([1, 256], F32, tag="bnp")
            nc.vector.tensor_tensor(bnp[:], B[0:1, 512:768],
                                    bhhn[0:1, ci * 256:(ci + 1) * 256], op=ALU.add)
            rb = rows.tile([1, 256], F32, tag="rb")
            nc.vector.tensor_tensor(rb[:], sig[0:1, 0:256], bnp[:], op=ALU.mult)
            nin = rows.tile([1, 256], F32, tag="nin")
            nc.vector.tensor_tensor(nin[:], A[0:1, 512:768],
                                    bihn[0:1, ci * 256:(ci + 1) * 256], op=ALU.add)
            nc.vector.tensor_tensor(nin[:], nin[:], rb[:], op=ALU.add)
            n_row = rows.tile([1, 256], F32, tag="n_row")
            nc.scalar.activation(n_row[:], nin[:], AF.Tanh)
            hmn = rows.tile([1, 256], F32, tag="hmn")
            nc.vector.tensor_tensor(hmn[:], h_row_ap, n_row[:], op=ALU.subtract)
            zm = rows.tile([1, 256], F32, tag="zm")
            nc.vector.tensor_tensor(zm[:], sig[0:1, 256:512], hmn[:], op=ALU.mult)
            nc.vector.tensor_tensor(out_row_ap, n_row[:], zm[:], op=ALU.add)

        xs_x = [x_col[:, 0:1], x_col[:, 1:2]]
        for dd in range(2):
            gru_cell(dd, xs_x, ih0_pm, dd * 1536,
                     [hcols[:, 2 * dd:2 * dd + 1], hcols[:, 2 * dd + 1:2 * dd + 2]],
                     hh0_pm, dd * 1536,
                     hrows[0:1, dd * H:(dd + 1) * H],
                     hid_out[0:1, dd * H:(dd + 1) * H])

        out0_col = row2col(hid_out[0:1, 0:512], 4, "out0_col")
        xs_o0 = [out0_col[:, b:b + 1] for b in range(4)]
        for dd in range(2):
            gru_cell(2 + dd, xs_o0, ih1_pm, dd * 3072,
                     [hcols[:, 4 + 2 * dd:5 + 2 * dd], hcols[:, 5 + 2 * dd:6 + 2 * dd]],
                     hh1_pm, dd * 1536,
                     hrows[0:1, (2 + dd) * H:(3 + dd) * H],
                     hid_out[0:1, (2 + dd) * H:(3 + dd) * H])

        nc.sync.dma_start(d["nh_out"][:], hid_out[:])
        out1_col = row2col(hid_out[0:1, 512:1024], 4, "out1_col")

        # warm the exp/ln table set during the big matvec; input spans both
        # layer-1 outputs so it lands after every tanh in ACT program order.
        scr2 = rows.tile([1, 1], F32, tag="scr2")
        nc.scalar.activation(scr2[:], hid_out[0:1, 767:768], AF.Ln,
                             bias=1.0, scale=0.0)

        # ---- big matvec: logits shard (bias-less), streamed to DRAM ----
        ag_in = dram.tile([1, VS], F32, tag="ag_in")
        for g, gw in enumerate(NG_W):
            for j in range(gw // 512):
                ps = mvp.tile([1, 512], F32, tag="mv")
                for b in range(4):
                    nc.tensor.matmul(ps[:], lhsT=out1_col[:, b:b + 1],
                                     rhs=wslab[(b, g)][:, j * 512:(j + 1) * 512],
                                     start=(b == 0), stop=(b == 3))
                lg = lgp.tile([1, 512], F32, tag="lg")
                nc.scalar.copy(lg[:], ps[:])
                n0 = goff[g] + j * 512
                nc.sync.dma_start(ag_in[0:1, n0:n0 + 512], lg[:])

        # ---- AllGather + log_softmax tail ----
        ag_out = dram.tile([NC, VS], F32, tag="ag_out")
        nc.gpsimd.collective_compute(
            "AllGather", ALU.bypass, replica_groups=[list(range(NC))],
            ins=[ag_in[:].opt()], outs=[ag_out[:].opt()])

        logits_pm = sb.tile([128, FW], F32, tag="logits_pm")
        nc.sync.dma_start(
            logits_pm[:],
            ag_out[:].rearrange("a b -> (a b)").rearrange("(p j) -> p j", p=128))
        nc.vector.tensor_tensor(logits_pm[:], logits_pm[:], outb_pm[:], op=ALU.add)

        exp_pm = sb.tile([128, FW], F32, tag="exp_pm")
        sums = sb.tile([128, 1], F32, tag="sums")
        nc.scalar.activation(exp_pm[:], logits_pm[:], AF.Exp, accum_out=sums[:])
        ps_s = cp.tile([1, 1], F32, tag="cps")
        nc.tensor.matmul(ps_s[:], lhsT=ones_col, rhs=sums[:], start=True, stop=True)
        lse = rows.tile([1, 1], F32, tag="lse")
        nc.scalar.activation(lse[:], ps_s[:], AF.Ln)
        ps_b = cp.tile([128, 1], F32, tag="cps")
        nc.tensor.matmul(ps_b[:], lhsT=ones_row, rhs=lse[:], start=True, stop=True)
        lse_col = sb.tile([128, 1], F32, tag="lse_col")
        nc.vector.tensor_copy(lse_col[:], ps_b[:])
        logp_pm = sb.tile([128, FW], F32, tag="logp_pm")
        nc.vector.tensor_scalar(logp_pm[:], logits_pm[:], lse_col[:, 0:1], None,
                                op0=ALU.subtract)
        nc.sync.dma_start(d["logp_out"][:], logp_pm[:])


def _stage(inputs):
    f32 = lambda a: np.ascontiguousarray(np.asarray(a, dtype=np.float32))
    hidden = f32(inputs["hidden"])
    b_ih0, b_hh0 = f32(inputs["b_ih0"]), f32(inputs["b_hh0"])
    b_ih1, b_hh1 = f32(inputs["b_ih1"]), f32(inputs["b_hh1"])
    out_W, out_b = f32(inputs["out_W"]), f32(inputs["out_b"])

    # cell order: l0c0, l0c1, l1c0, l1c1
    bih = [b_ih0[0], b_ih0[1], b_ih1[0], b_ih1[1]]
    bhh = [b_hh0[0], b_hh0[1], b_hh1[0], b_hh1[1]]
    brz_sum = np.concatenate([(bih[c][0:512] + bhh[c][0:512]) for c in range(4)])
    bihn = np.concatenate([bih[c][512:768] for c in range(4)])
    bhhn = np.concatenate([bhh[c][512:768] for c in range(4)])

    outb_pad = np.full(VPAD, NEG_BIG, np.float32)
    outb_pad[:V] = out_b

    consts = np.zeros((128, 132), np.float32)
    consts[:, 0] = 1.0
    consts[0, 4:132] = 1.0

    com = {
        "idx": np.asarray(inputs["input_ids"]).astype(np.int32).reshape(1, 1),
        "emb": f32(inputs["emb"]),
        "attnw_pm": _pm(np.ascontiguousarray(f32(inputs["attn_W"]).T)),
        "cv_pm": _pm(f32(inputs["context_vector"])),
        "comb_pm": _pm(np.ascontiguousarray(f32(inputs["comb_W"]).T)),
        "ih0_pm": np.concatenate(
            [_pm(np.ascontiguousarray(f32(inputs["w_ih0"])[d].T)) for d in range(2)], axis=1),
        "hh0_pm": np.concatenate(
            [_pm(np.ascontiguousarray(f32(inputs["w_hh0"])[d].T)) for d in range(2)], axis=1),
        "ih1_pm": np.concatenate(
            [_pm(np.ascontiguousarray(f32(inputs["w_ih1"])[d].T)) for d in range(2)], axis=1),
        "hh1_pm": np.concatenate(
            [_pm(np.ascontiguousarray(f32(inputs["w_hh1"])[d].T)) for d in range(2)], axis=1),
        "attnb": f32(inputs["attn_b"]).reshape(1, L),
        "combb": f32(inputs["comb_b"]).reshape(1, H),
        "brz_sum": brz_sum.reshape(1, 4 * 512),
        "bihn": bihn.reshape(1, 4 * 256),
        "bhhn": bhhn.reshape(1, 4 * 256),
        "hidden_cols": np.ascontiguousarray(np.concatenate(
            [hidden[dd, 0].reshape(2, 128).T for dd in range(4)], axis=1)),
        "hidden_rows": np.ascontiguousarray(hidden.reshape(1, 4 * H)),
        "consts": consts,
        "outb_pm": np.ascontiguousarray(outb_pad.reshape(128, FW)),
    }
    for k in com:
        com[k] = np.ascontiguousarray(com[k])

    in_maps = []
    for c in range(NC):
        lo, hi = c * VS, min(c * VS + VS, V)
        wsh = np.zeros((VS, 512), np.float32)
        if hi > lo:
            wsh[: hi - lo] = out_W[lo:hi]
        m = dict(com)
        m["outw_pm"] = _pm(np.ascontiguousarray(wsh.T))
        in_maps.append(m)
    return in_maps


def kernel(**inputs):
    global LAST_EXEC_NS
    if "nc" not in _CACHE:
        _CACHE["nc"] = _build_nc()
    in_maps = _stage(inputs)
    res = run_bass_kernel_spmd(_CACHE["nc"], in_maps, core_ids=list(range(NC)))
    LAST_EXEC_NS = res.exec_time_ns
    _CACHE["last_results"] = res
    r0 = res.results[0]
    logp = r0["logp_out"].reshape(VPAD)[:V].reshape(1, V).astype(np.float32)
    nh = r0["nh_out"].reshape(4, 1, H).astype(np.float32)
    aw = r0["aw_out"].reshape(1, L).astype(np.float32)
    return logp, nh, aw


# revision 8
# speedup vs baseline: 1.5453x; 1.5453x over previous
"""Trainium2 Bass kernel for nn_AttnDecoderRNN (batch=1 single-step decode).

Vocab/tensor-parallel per the sharding hint:
  - out_W is sharded across 8 cores on the vocab dim (bf16, host
    pre-swizzled partition-major); each core computes its logits shard on
    the TensorEngine, a 32-byte AllGather shares per-core sumexp scalars,
    and each core writes its own logp shard (host concatenates).
  - The small recurrent chain is replicated on every core. All matvecs run
    on the TensorEngine in row form with K-block lhsT columns; biases are
    folded in as K=1 matmuls accumulating into the same PSUM tile, so the
    GRU gate math is 7 DVE/ACT ops per cell.
"""

import numpy as np

import concourse.bass as bass
import concourse.bacc as bacc
import concourse.mybir as mybir
import concourse.tile as tile
from concourse.bass_utils import run_bass_kernel_spmd

F32 = mybir.dt.float32
BF16 = mybir.dt.bfloat16
I32 = mybir.dt.int32
AF = mybir.ActivationFunctionType
ALU = mybir.AluOpType

H = 256
L = 512
V = 50257
NC = 8
VS = 6656          # padded vocab shard per core (13*512, 52*128)
SW = VS // 128     # 52
NG_W = [4096, 2560]
NEG_BIG = -1.0e4

# offsets inside the packed tensors
WC_ATTN, WC_CV, WC_COMB, WC_HCOL = 0, 2048, 4096, 5632   # wchain cols
WG_IH0, WG_HH0, WG_IH1, WG_HH1 = 0, 3072, 6144, 12288    # wgru cols
BB_ATT, BB_COMB, BB_GRU = 0, 512, 768                     # bias_bf cols

LAST_EXEC_NS = None
_CACHE = {}


def _pm(m):
    """[K, N] -> [128, (K//128)*N] with pm[p, b*N+n] = m[b*128+p, n]."""
    k, n = m.shape
    assert k % 128 == 0
    return np.ascontiguousarray(
        m.reshape(k // 128, 128, n).transpose(1, 0, 2).reshape(128, (k // 128) * n))


def _build_nc():
    nc = bacc.Bacc("TRN2", target_bir_lowering=False, debug=False,
                   enable_asserts=False, num_devices=NC)
    d = {}
    def inp(name, shape, dt=F32):
        d[name] = nc.dram_tensor(name, shape, dt, kind="ExternalInput").ap()
    inp("idx2", [2, 1], I32)
    inp("emb", [V, H])
    inp("consts2", [128, 184])          # ones/ones_row + outb_pm shard
    inp("frow", [1, 4 * H])             # hidden rows
    inp("bias_bf", [1, 8 * 768 + 768], BF16)
    inp("wchain", [128, 5640], BF16)    # attnw | cv | comb | hcols
    inp("wgru", [128, 15360], BF16)     # ih0 | hh0 | ih1 | hh1
    inp("outw_pm", [128, 4 * VS], BF16)
    d["logp_out"] = nc.dram_tensor("logp_out", [128, SW], F32,
                                   kind="ExternalOutput").ap()
    d["nh_out"] = nc.dram_tensor("nh_out", [1, 4 * H], F32,
                                 kind="ExternalOutput").ap()
    d["aw_out"] = nc.dram_tensor("aw_out", [1, L], F32,
                                 kind="ExternalOutput").ap()
    with tile.TileContext(nc) as tc:
        _emit(nc, tc, d)
    nc.compile()
    return nc


def _emit(nc, tc, d):
    from contextlib import ExitStack
    with ExitStack() as ctx:
        sb = ctx.enter_context(tc.tile_pool(name="sb", bufs=1))
        wpool = ctx.enter_context(tc.tile_pool(name="wpool", bufs=8))
        lgp = ctx.enter_context(tc.tile_pool(name="lgp", bufs=3))
        rows = ctx.enter_context(tc.tile_pool(name="rows", bufs=1))
        mvp = ctx.enter_context(tc.tile_pool(name="mvp", bufs=5, space="PSUM"))
        gp = ctx.enter_context(tc.tile_pool(name="gp", bufs=1, space="PSUM"))
        cp = ctx.enter_context(tc.tile_pool(name="cp", bufs=1, space="PSUM"))
        dram = ctx.enter_context(tc.tile_pool(name="dram", bufs=1, space="DRAM"))

        one_bf = nc.const_aps.tensor(1.0, (1, 1), BF16)

        # ---- input loads (order = priority) ----
        idx2 = sb.tile([2, 1], I32, tag="idx2")
        nc.sync.dma_start(idx2[:], d["idx2"][:])
        consts2 = sb.tile([128, 184], F32, tag="consts2")
        nc.sync.dma_start(consts2[:], d["consts2"][:])
        wchain = sb.tile([128, 5640], BF16, tag="wchain")
        nc.sync.dma_start(wchain[:], d["wchain"][:])
        bias_bf = sb.tile([1, 8 * 768 + 768], BF16, tag="bias_bf")
        nc.sync.dma_start(bias_bf[:], d["bias_bf"][:])
        frow = sb.tile([1, 4 * H], F32, tag="frow")
        nc.sync.dma_start(frow[:], d["frow"][:])
        wgru = sb.tile([128, 15360], BF16, tag="wgru")
        nc.sync.dma_start(wgru[:], d["wgru"][:])

        ones_col = consts2[:, 0:1]
        ones_row = consts2[0:1, 4:132]
        outb_pm = consts2[:, 132:184]
        hcols = wchain[:, WC_HCOL:WC_HCOL + 8]

        emb2 = sb.tile([2, H], F32, tag="emb2")
        nc.gpsimd.indirect_dma_start(
            out=emb2[:], out_offset=None, in_=d["emb"][:],
            in_offset=bass.IndirectOffsetOnAxis(ap=idx2[:, 0:1], axis=0))
        emb_row = emb2[0:1, :]

        # out_W slabs on the scalar HWDGE queue (sync queue stays free)
        goff = [0, NG_W[0]]
        wslab = {}
        for g, gw in enumerate(NG_W):
            for b in range(4):
                t = wpool.tile([128, NG_W[0]], BF16, tag="wsl")
                nc.scalar.dma_start(
                    t[:, 0:gw],
                    d["outw_pm"][:, b * VS + goff[g]: b * VS + goff[g] + gw])
                wslab[(b, g)] = t

        # warm the exp table set immediately
        scr1 = rows.tile([1, 1], F32, tag="scr1")
        nc.scalar.activation(scr1[:], consts2[0:1, 0:1], AF.Exp)

        # ---- helpers ----
        def mv(ps_ap, groups, biases, n0, n1):
            """psum = sum_g sum_b xs[b].T @ w[:, base+b*n_tot+n0:n1] + biases."""
            total = sum(len(xs) for *_, xs in groups) + len(biases)
            k = 0
            for w_tile, base, n_tot, xs in groups:
                for b, xb in enumerate(xs):
                    nc.tensor.matmul(
                        ps_ap, lhsT=xb,
                        rhs=w_tile[:, base + b * n_tot + n0: base + b * n_tot + n1],
                        start=(k == 0), stop=(k == total - 1))
                    k += 1
            for btile, boff in biases:
                nc.tensor.matmul(ps_ap, lhsT=one_bf,
                                 rhs=btile[0:1, boff + n0: boff + n1],
                                 start=(k == 0), stop=(k == total - 1))
                k += 1

        def row2col(row_ap, nchunks, dst_tag, is_bf16):
            if not is_bf16:
                rb = rows.tile([1, 128 * nchunks], BF16, tag="r2c_" + dst_tag)
                nc.vector.tensor_copy(rb[:], row_ap)
                row_ap = rb[:]
            ps = cp.tile([128, 4], F32, tag="cps")
            for c in range(nchunks):
                nc.tensor.matmul(ps[:, c:c + 1],
                                 lhsT=row_ap[0:1, c * 128:(c + 1) * 128],
                                 rhs=one_bf, start=True, stop=True)
            col = sb.tile([128, nchunks], BF16, tag=dst_tag)
            nc.vector.tensor_copy(col[:], ps[:, 0:nchunks])
            return col

        # ---- attention ----
        emb_col = row2col(emb_row, 2, "emb_col", False)
        xs_attn = [emb_col[:, 0:1], emb_col[:, 1:2], hcols[:, 0:1], hcols[:, 1:2]]
        ps_attn = gp.tile([1, 512], F32, tag="ga")
        mv(ps_attn[:], [(wchain, WC_ATTN, L, xs_attn)], [(bias_bf, BB_ATT)], 0, L)
        expw = rows.tile([1, L], F32, tag="expw")
        ssum = rows.tile([1, 1], F32, tag="ssum")
        nc.scalar.activation(expw[:], ps_attn[:], AF.Exp, accum_out=ssum[:])
        # switch ACT to the sigmoid/tanh set while CV+comb matvecs run
        scr2 = rows.tile([1, 1], F32, tag="scr2")
        nc.scalar.activation(scr2[:], expw[0:1, 0:1], AF.Sigmoid,
                             bias=1.0, scale=0.0)
        rsum = rows.tile([1, 1], F32, tag="rsum")
        nc.vector.reciprocal(rsum[:], ssum[:])
        aw_row = rows.tile([1, L], F32, tag="aw_row")
        nc.vector.tensor_scalar(aw_row[:], expw[:], rsum[:, 0:1], None,
                                op0=ALU.mult)
        nc.sync.dma_start(d["aw_out"][:], aw_row[:])

        aw_col = row2col(aw_row[:], 4, "aw_col", False)
        ps_app = gp.tile([1, 512], F32, tag="gb")
        mv(ps_app[:], [(wchain, WC_CV, L,
                        [aw_col[:, b:b + 1] for b in range(4)])], [], 0, L)
        app_row = rows.tile([1, L], BF16, tag="app_row")
        nc.scalar.copy(app_row[:], ps_app[:])
        app_col = row2col(app_row[:], 4, "app_col", True)

        # ---- comb + relu ----
        xs_comb = [emb_col[:, 0:1], emb_col[:, 1:2]] + \
                  [app_col[:, b:b + 1] for b in range(4)]
        ps_x = gp.tile([1, 512], F32, tag="ga")
        mv(ps_x[0:1, 0:H], [(wchain, WC_COMB, H, xs_comb)],
           [(bias_bf, BB_COMB)], 0, H)
        x_row = rows.tile([1, H], BF16, tag="x_row")
        nc.scalar.activation(x_row[:], ps_x[0:1, 0:H], AF.Relu)
        x_col = row2col(x_row[:], 2, "x_col", True)

        # ---- GRU cells (7 DVE/ACT ops each; biases folded into PSUM) ----
        hid_out = rows.tile([1, 4 * H], F32, tag="hid_out")

        def gru_cell(ci, xs, ih_base, h_cols, hh_base, h_row_ap, out_row_ap):
            bih = BB_GRU + ci * 768
            bhh = BB_GRU + (4 + ci) * 768
            t_rz = gp.tile([1, 512], F32, tag="ga")
            mv(t_rz[:], [(wgru, ih_base, 768, xs), (wgru, hh_base, 768, h_cols)],
               [(bias_bf, bih), (bias_bf, bhh)], 0, 512)
            t_n = gp.tile([1, 512], F32, tag="gb")
            mv(t_n[0:1, 0:256], [(wgru, ih_base, 768, xs)],
               [(bias_bf, bih)], 512, 768)
            mv(t_n[0:1, 256:512], [(wgru, hh_base, 768, h_cols)],
               [(bias_bf, bhh)], 512, 768)
            sig = rows.tile([1, 512], F32, tag="sig")
            nc.scalar.activation(sig[:], t_rz[:], AF.Sigmoid)
            rb = rows.tile([1, 256], F32, tag="rb")
            nc.vector.tensor_tensor(rb[:], sig[0:1, 0:256], t_n[0:1, 256:512],
                                    op=ALU.mult)
            nin = rows.tile([1, 256], F32, tag="nin")
            nc.vector.tensor_tensor(nin[:], t_n[0:1, 0:256], rb[:], op=ALU.add)
            n_row = rows.tile([1, 256], F32, tag="n_row")
            nc.scalar.activation(n_row[:], nin[:], AF.Tanh)
            hmn = rows.tile([1, 256], F32, tag="hmn")
            nc.vector.tensor_tensor(hmn[:], h_row_ap, n_row[:], op=ALU.subtract)
            zm = rows.tile([1, 256], F32, tag="zm")
            nc.vector.tensor_tensor(zm[:], sig[0:1, 256:512], hmn[:], op=ALU.mult)
            nc.vector.tensor_tensor(out_row_ap, n_row[:], zm[:], op=ALU.add)

        xs_x = [x_col[:, 0:1], x_col[:, 1:2]]
        for dd in range(2):
            gru_cell(dd, xs_x, WG_IH0 + dd * 1536,
                     [hcols[:, 2 * dd:2 * dd + 1], hcols[:, 2 * dd + 1:2 * dd + 2]],
                     WG_HH0 + dd * 1536,
                     frow[0:1, dd * H:(dd + 1) * H],
                     hid_out[0:1, dd * H:(dd + 1) * H])
        out0_col = row2col(hid_out[0:1, 0:512], 4, "out0_col", False)
        xs_o0 = [out0_col[:, b:b + 1] for b in range(4)]
        for dd in range(2):
            gru_cell(2 + dd, xs_o0, WG_IH1 + dd * 3072,
                     [hcols[:, 4 + 2 * dd:5 + 2 * dd], hcols[:, 5 + 2 * dd:6 + 2 * dd]],
                     WG_HH1 + dd * 1536,
                     frow[0:1, (2 + dd) * H:(3 + dd) * H],
                     hid_out[0:1, (2 + dd) * H:(3 + dd) * H])

        nc.sync.dma_start(d["nh_out"][:], hid_out[:])
        out1_col = row2col(hid_out[0:1, 512:1024], 4, "out1_col", False)

        # switch ACT to the exp/ln set during the big matvec
        scr3 = rows.tile([1, 1], F32, tag="scr3")
        nc.scalar.activation(scr3[:], hid_out[0:1, 767:768], AF.Ln,
                             bias=1.0, scale=0.0)

        # ---- big matvec: 13 N-tiles of 512, streamed to ag_in ----
        ag_in = dram.tile([1, VS], F32, tag="ag_in")
        for g, gw in enumerate(NG_W):
            for j in range(gw // 512):
                ps = mvp.tile([1, 512], F32, tag="mv")
                for b in range(4):
                    nc.tensor.matmul(ps[:], lhsT=out1_col[:, b:b + 1],
                                     rhs=wslab[(b, g)][:, j * 512:(j + 1) * 512],
                                     start=(b == 0), stop=(b == 3))
                lg = lgp.tile([1, 512], F32, tag="lg")
                nc.scalar.copy(lg[:], ps[:])
                n0 = goff[g] + j * 512
                nc.sync.dma_start(ag_in[0:1, n0:n0 + 512], lg[:])

        # ---- tail: local sumexp, 32B AllGather, logp shard ----
        lpm = sb.tile([128, SW], F32, tag="lpm")
        nc.sync.dma_start(lpm[:], ag_in[:].rearrange("a (p j) -> (a p) j", p=128))
        nc.vector.tensor_tensor(lpm[:], lpm[:], outb_pm, op=ALU.add)
        exp_pm = sb.tile([128, SW], F32, tag="exp_pm")
        sums = sb.tile([128, 1], F32, tag="sums")
        nc.scalar.activation(exp_pm[:], lpm[:], AF.Exp, accum_out=sums[:])
        ps_s = cp.tile([1, 1], F32, tag="cps")
        nc.tensor.matmul(ps_s[:], lhsT=ones_col, rhs=sums[:], start=True, stop=True)
        s_sb = rows.tile([1, 1], F32, tag="s_sb")
        nc.vector.tensor_copy(s_sb[:], ps_s[:])
        s_in = dram.tile([1, 1], F32, tag="s_in")
        s_out = dram.tile([1, NC], F32, tag="s_out")
        nc.sync.dma_start(s_in[:], s_sb[:])
        nc.gpsimd.collective_compute(
            "AllGather", ALU.bypass, replica_groups=[list(range(NC))],
            ins=[s_in[:].opt()], outs=[s_out[:].opt()])
        s_row = rows.tile([1, NC], F32, tag="s_row")
        nc.sync.dma_start(s_row[:], s_out[:])
        stot = rows.tile([1, 1], F32, tag="stot")
        nc.vector.reduce_sum(stot[:], s_row[:], axis=mybir.AxisListType.X)
        lse = rows.tile([1, 1], F32, tag="lse")
        nc.scalar.activation(lse[:], stot[:], AF.Ln)
        ps_b = cp.tile([128, 1], F32, tag="cps")
        nc.tensor.matmul(ps_b[:], lhsT=ones_row, rhs=lse[:], start=True, stop=True)
        lse_col = sb.tile([128, 1], F32, tag="lse_col")
        nc.vector.tensor_copy(lse_col[:], ps_b[:])
        logp_pm = sb.tile([128, SW], F32, tag="logp_pm")
        nc.vector.tensor_scalar(logp_pm[:], lpm[:], lse_col[:, 0:1], None,
                                op0=ALU.subtract)
        nc.sync.dma_start(d["logp_out"][:], logp_pm[:])


def _stage(inputs):
    import ml_dtypes
    bf16 = ml_dtypes.bfloat16
    f32 = lambda a: np.ascontiguousarray(np.asarray(a, dtype=np.float32))
    hidden = f32(inputs["hidden"])
    out_W, out_b = f32(inputs["out_W"]), f32(inputs["out_b"])
    w_ih0, w_hh0 = f32(inputs["w_ih0"]), f32(inputs["w_hh0"])
    w_ih1, w_hh1 = f32(inputs["w_ih1"]), f32(inputs["w_hh1"])
    b_ih0, b_hh0 = f32(inputs["b_ih0"]), f32(inputs["b_hh0"])
    b_ih1, b_hh1 = f32(inputs["b_ih1"]), f32(inputs["b_hh1"])

    consts2 = np.zeros((128, 184), np.float32)
    consts2[:, 0] = 1.0
    consts2[0, 4:132] = 1.0
    outb_pad = np.full(VS * NC, NEG_BIG, np.float32)
    outb_pad[:V] = out_b

    # bias_bf: attnb | combb | bih(c0..c3) | bhh(c0..c3)
    bih = [b_ih0[0], b_ih0[1], b_ih1[0], b_ih1[1]]
    bhh = [b_hh0[0], b_hh0[1], b_hh1[0], b_hh1[1]]
    bias_bf = np.concatenate(
        [f32(inputs["attn_b"]).reshape(L), f32(inputs["comb_b"]).reshape(H)]
        + bih + bhh).astype(bf16).reshape(1, -1)
    assert bias_bf.shape[1] == 768 + 8 * 768

    hcols = np.concatenate(
        [hidden[dd, 0].reshape(2, 128).T for dd in range(4)], axis=1)
    wchain = np.concatenate([
        _pm(np.ascontiguousarray(f32(inputs["attn_W"]).T)),
        _pm(f32(inputs["context_vector"])),
        _pm(np.ascontiguousarray(f32(inputs["comb_W"]).T)),
        hcols], axis=1).astype(bf16)
    assert wchain.shape == (128, 5640)
    wgru = np.concatenate(
        [_pm(np.ascontiguousarray(w_ih0[dd].T)) for dd in range(2)]
        + [_pm(np.ascontiguousarray(w_hh0[dd].T)) for dd in range(2)]
        + [_pm(np.ascontiguousarray(w_ih1[dd].T)) for dd in range(2)]
        + [_pm(np.ascontiguousarray(w_hh1[dd].T)) for dd in range(2)],
        axis=1).astype(bf16)
    assert wgru.shape == (128, 15360)

    idx = int(np.asarray(inputs["input_ids"]).reshape(-1)[0])
    com = {
        "idx2": np.full((2, 1), idx, np.int32),
        "emb": f32(inputs["emb"]),
        "frow": np.ascontiguousarray(hidden.reshape(1, 4 * H)),
        "bias_bf": np.ascontiguousarray(bias_bf),
        "wchain": np.ascontiguousarray(wchain),
        "wgru": np.ascontiguousarray(wgru),
    }
    in_maps = []
    for c in range(NC):
        lo, hi = c * VS, min(c * VS + VS, V)
        wsh = np.zeros((VS, 512), np.float32)
        if hi > lo:
            wsh[: hi - lo] = out_W[lo:hi]
        m = dict(com)
        m["outw_pm"] = _pm(np.ascontiguousarray(wsh.T)).astype(bf16)
        cc = consts2.copy()
        cc[:, 132:184] = outb_pad[lo:lo + VS].reshape(128, SW)
        m["consts2"] = cc
        in_maps.append(m)
    return in_maps


def kernel(**inputs):
    global LAST_EXEC_NS
    if "nc" not in _CACHE:
        _CACHE["nc"] = _build_nc()
    in_maps = _stage(inputs)
    res = run_bass_kernel_spmd(_CACHE["nc"], in_maps, core_ids=list(range(NC)))
    LAST_EXEC_NS = res.exec_time_ns
    _CACHE["last_results"] = res
    r0 = res.results[0]
    logp = np.concatenate(
        [res.results[c]["logp_out"].reshape(VS) for c in range(NC)])[:V]
    logp = logp.reshape(1, V).astype(np.float32)
    nh = r0["nh_out"].reshape(4, 1, H).astype(np.float32)
    aw = r0["aw_out"].reshape(1, L).astype(np.float32)
    return logp, nh, aw


# revision 9
# speedup vs baseline: 1.5470x; 1.0011x over previous
"""Trainium2 Bass kernel for nn_AttnDecoderRNN (batch=1 single-step decode).

Vocab/tensor-parallel per the sharding hint:
  - out_W is sharded across 8 cores on the vocab dim (bf16, host
    pre-swizzled partition-major); each core computes its logits shard on
    the TensorEngine, a 32-byte AllGather shares per-core sumexp scalars,
    and each core writes its own logp shard (host concatenates).
  - The small recurrent chain is replicated on every core. All matvecs run
    on the TensorEngine in row form with K-block lhsT columns; biases are
    folded in as K=1 matmuls accumulating into the same PSUM tile, so the
    GRU gate math is 7 DVE/ACT ops per cell.
"""

import numpy as np

import concourse.bass as bass
import concourse.bacc as bacc
import concourse.mybir as mybir
import concourse.tile as tile
from concourse.bass_utils import run_bass_kernel_spmd

F32 = mybir.dt.float32
BF16 = mybir.dt.bfloat16
I32 = mybir.dt.int32
AF = mybir.ActivationFunctionType
ALU = mybir.AluOpType

H = 256
L = 512
V = 50257
NC = 8
VS = 6656          # padded vocab shard per core (13*512, 52*128)
SW = VS // 128     # 52
NG_W = [4096, 2560]
NEG_BIG = -1.0e4

# offsets inside the packed tensors
WC_ATTN, WC_CV, WC_COMB, WC_HCOL = 0, 2048, 4096, 5632   # wchain cols
WG_IH0, WG_HH0, WG_IH1, WG_HH1 = 0, 3072, 6144, 12288    # wgru cols
BB_ATT, BB_COMB, BB_GRU = 0, 512, 768                     # bias_bf cols

LAST_EXEC_NS = None
_CACHE = {}


def _pm(m):
    """[K, N] -> [128, (K//128)*N] with pm[p, b*N+n] = m[b*128+p, n]."""
    k, n = m.shape
    assert k % 128 == 0
    return np.ascontiguousarray(
        m.reshape(k // 128, 128, n).transpose(1, 0, 2).reshape(128, (k // 128) * n))


def _build_nc():
    nc = bacc.Bacc("TRN2", target_bir_lowering=False, debug=False,
                   enable_asserts=False, num_devices=NC)
    d = {}
    def inp(name, shape, dt=F32):
        d[name] = nc.dram_tensor(name, shape, dt, kind="ExternalInput").ap()
    inp("idx2", [2, 1], I32)
    inp("emb", [V, H])
    inp("consts2", [128, 184])          # ones/ones_row + outb_pm shard
    inp("frow", [1, 4 * H])             # hidden rows
    inp("bias_bf", [1, 8 * 768 + 768], BF16)
    inp("wchain", [128, 5640], BF16)    # attnw | cv | comb | hcols
    inp("wgru", [128, 15360], BF16)     # ih0 | hh0 | ih1 | hh1
    inp("outw_pm", [128, 4 * VS], BF16)
    d["logp_out"] = nc.dram_tensor("logp_out", [128, SW], F32,
                                   kind="ExternalOutput").ap()
    d["nh_out"] = nc.dram_tensor("nh_out", [1, 4 * H], F32,
                                 kind="ExternalOutput").ap()
    d["aw_out"] = nc.dram_tensor("aw_out", [1, L], F32,
                                 kind="ExternalOutput").ap()
    with tile.TileContext(nc) as tc:
        _emit(nc, tc, d)
    nc.compile()
    return nc


def _emit(nc, tc, d):
    from contextlib import ExitStack
    with ExitStack() as ctx:
        sb = ctx.enter_context(tc.tile_pool(name="sb", bufs=1))
        wpool = ctx.enter_context(tc.tile_pool(name="wpool", bufs=8))
        lgp = ctx.enter_context(tc.tile_pool(name="lgp", bufs=3))
        rows = ctx.enter_context(tc.tile_pool(name="rows", bufs=1))
        mvp = ctx.enter_context(tc.tile_pool(name="mvp", bufs=5, space="PSUM"))
        gp = ctx.enter_context(tc.tile_pool(name="gp", bufs=1, space="PSUM"))
        cp = ctx.enter_context(tc.tile_pool(name="cp", bufs=1, space="PSUM"))
        dram = ctx.enter_context(tc.tile_pool(name="dram", bufs=1, space="DRAM"))

        one_bf = nc.const_aps.tensor(1.0, (1, 1), BF16)

        # ---- input loads (order = priority) ----
        idx2 = sb.tile([2, 1], I32, tag="idx2")
        nc.sync.dma_start(idx2[:], d["idx2"][:])
        consts2 = sb.tile([128, 184], F32, tag="consts2")
        nc.sync.dma_start(consts2[:], d["consts2"][:])
        wchain = sb.tile([128, 5640], BF16, tag="wchain")
        nc.sync.dma_start(wchain[:], d["wchain"][:])
        bias_bf = sb.tile([1, 8 * 768 + 768], BF16, tag="bias_bf")
        nc.sync.dma_start(bias_bf[:], d["bias_bf"][:])
        frow = sb.tile([1, 4 * H], F32, tag="frow")
        nc.sync.dma_start(frow[:], d["frow"][:])
        wgru = sb.tile([128, 15360], BF16, tag="wgru")
        nc.sync.dma_start(wgru[:], d["wgru"][:])

        ones_col = consts2[:, 0:1]
        ones_row = consts2[0:1, 4:132]
        outb_pm = consts2[:, 132:184]
        hcols = wchain[:, WC_HCOL:WC_HCOL + 8]

        emb2 = sb.tile([2, H], F32, tag="emb2")
        nc.gpsimd.indirect_dma_start(
            out=emb2[:], out_offset=None, in_=d["emb"][:],
            in_offset=bass.IndirectOffsetOnAxis(ap=idx2[:, 0:1], axis=0))
        emb_row = emb2[0:1, :]

        # out_W slabs on the scalar HWDGE queue (sync queue stays free)
        goff = [0, NG_W[0]]
        wslab = {}
        for g, gw in enumerate(NG_W):
            for b in range(4):
                t = wpool.tile([128, NG_W[0]], BF16, tag="wsl")
                nc.scalar.dma_start(
                    t[:, 0:gw],
                    d["outw_pm"][:, b * VS + goff[g]: b * VS + goff[g] + gw])
                wslab[(b, g)] = t

        # warm the exp table set immediately
        scr1 = rows.tile([1, 1], F32, tag="scr1")
        nc.scalar.activation(scr1[:], consts2[0:1, 0:1], AF.Exp)

        # ---- helpers ----
        def mv(ps_ap, groups, biases, n0, n1):
            """psum = sum_g sum_b xs[b].T @ w[:, base+b*n_tot+n0:n1] + biases."""
            total = sum(len(xs) for *_, xs in groups) + len(biases)
            k = 0
            for w_tile, base, n_tot, xs in groups:
                for b, xb in enumerate(xs):
                    nc.tensor.matmul(
                        ps_ap, lhsT=xb,
                        rhs=w_tile[:, base + b * n_tot + n0: base + b * n_tot + n1],
                        start=(k == 0), stop=(k == total - 1))
                    k += 1
            for btile, boff in biases:
                nc.tensor.matmul(ps_ap, lhsT=one_bf,
                                 rhs=btile[0:1, boff + n0: boff + n1],
                                 start=(k == 0), stop=(k == total - 1))
                k += 1

        def row2col(row_ap, nchunks, dst_tag, is_bf16):
            if not is_bf16:
                rb = rows.tile([1, 128 * nchunks], BF16, tag="r2c_" + dst_tag)
                nc.vector.tensor_copy(rb[:], row_ap)
                row_ap = rb[:]
            ps = cp.tile([128, 4], F32, tag="cps")
            for c in range(nchunks):
                nc.tensor.matmul(ps[:, c:c + 1],
                                 lhsT=row_ap[0:1, c * 128:(c + 1) * 128],
                                 rhs=one_bf, start=True, stop=True)
            col = sb.tile([128, nchunks], BF16, tag=dst_tag)
            nc.vector.tensor_copy(col[:], ps[:, 0:nchunks])
            return col

        # ---- attention ----
        emb_col = row2col(emb_row, 2, "emb_col", False)
        xs_attn = [emb_col[:, 0:1], emb_col[:, 1:2], hcols[:, 0:1], hcols[:, 1:2]]
        ps_attn = gp.tile([1, 512], F32, tag="ga")
        mv(ps_attn[:], [(wchain, WC_ATTN, L, xs_attn)], [(bias_bf, BB_ATT)], 0, L)
        expw = rows.tile([1, L], F32, tag="expw")
        ssum = rows.tile([1, 1], F32, tag="ssum")
        nc.scalar.activation(expw[:], ps_attn[:], AF.Exp, accum_out=ssum[:])
        # switch ACT to the sigmoid/tanh set while CV+comb matvecs run
        scr2 = rows.tile([1, 1], F32, tag="scr2")
        nc.scalar.activation(scr2[:], expw[0:1, 0:1], AF.Sigmoid,
                             bias=1.0, scale=0.0)
        rsum = rows.tile([1, 1], F32, tag="rsum")
        nc.vector.reciprocal(rsum[:], ssum[:])
        aw_row = rows.tile([1, L], F32, tag="aw_row")
        nc.vector.tensor_scalar(aw_row[:], expw[:], rsum[:, 0:1], None,
                                op0=ALU.mult)
        nc.sync.dma_start(d["aw_out"][:], aw_row[:])

        aw_col = row2col(aw_row[:], 4, "aw_col", False)
        ps_app = gp.tile([1, 512], F32, tag="gb")
        mv(ps_app[:], [(wchain, WC_CV, L,
                        [aw_col[:, b:b + 1] for b in range(4)])], [], 0, L)
        app_row = rows.tile([1, L], BF16, tag="app_row")
        nc.scalar.copy(app_row[:], ps_app[:])
        app_col = row2col(app_row[:], 4, "app_col", True)

        # ---- comb + relu ----
        xs_comb = [emb_col[:, 0:1], emb_col[:, 1:2]] + \
                  [app_col[:, b:b + 1] for b in range(4)]
        ps_x = gp.tile([1, 512], F32, tag="ga")
        mv(ps_x[0:1, 0:H], [(wchain, WC_COMB, H, xs_comb)],
           [(bias_bf, BB_COMB)], 0, H)
        x_row = rows.tile([1, H], BF16, tag="x_row")
        nc.scalar.activation(x_row[:], ps_x[0:1, 0:H], AF.Relu)
        x_col = row2col(x_row[:], 2, "x_col", True)

        # ---- GRU cells (7 DVE/ACT ops each; biases folded into PSUM) ----
        hid_out = rows.tile([1, 4 * H], F32, tag="hid_out")

        def gru_cell(ci, xs, ih_base, h_cols, hh_base, h_row_ap, out_row_ap):
            bih = BB_GRU + ci * 768
            bhh = BB_GRU + (4 + ci) * 768
            t_rz = gp.tile([1, 512], F32, tag="ga")
            mv(t_rz[:], [(wgru, ih_base, 768, xs), (wgru, hh_base, 768, h_cols)],
               [(bias_bf, bih), (bias_bf, bhh)], 0, 512)
            t_n = gp.tile([1, 512], F32, tag="gb")
            mv(t_n[0:1, 0:256], [(wgru, ih_base, 768, xs)],
               [(bias_bf, bih)], 512, 768)
            mv(t_n[0:1, 256:512], [(wgru, hh_base, 768, h_cols)],
               [(bias_bf, bhh)], 512, 768)
            sig = rows.tile([1, 512], F32, tag="sig")
            nc.scalar.activation(sig[:], t_rz[:], AF.Sigmoid)
            rb = rows.tile([1, 256], F32, tag="rb")
            nc.vector.tensor_tensor(rb[:], sig[0:1, 0:256], t_n[0:1, 256:512],
                                    op=ALU.mult)
            nin = rows.tile([1, 256], F32, tag="nin")
            nc.vector.tensor_tensor(nin[:], t_n[0:1, 0:256], rb[:], op=ALU.add)
            n_row = rows.tile([1, 256], F32, tag="n_row")
            nc.scalar.activation(n_row[:], nin[:], AF.Tanh)
            hmn = rows.tile([1, 256], F32, tag="hmn")
            nc.vector.tensor_tensor(hmn[:], h_row_ap, n_row[:], op=ALU.subtract)
            zm = rows.tile([1, 256], F32, tag="zm")
            nc.vector.tensor_tensor(zm[:], sig[0:1, 256:512], hmn[:], op=ALU.mult)
            nc.vector.tensor_tensor(out_row_ap, n_row[:], zm[:], op=ALU.add)

        xs_x = [x_col[:, 0:1], x_col[:, 1:2]]
        for dd in range(2):
            gru_cell(dd, xs_x, WG_IH0 + dd * 1536,
                     [hcols[:, 2 * dd:2 * dd + 1], hcols[:, 2 * dd + 1:2 * dd + 2]],
                     WG_HH0 + dd * 1536,
                     frow[0:1, dd * H:(dd + 1) * H],
                     hid_out[0:1, dd * H:(dd + 1) * H])
        out0_col = row2col(hid_out[0:1, 0:512], 4, "out0_col", False)
        xs_o0 = [out0_col[:, b:b + 1] for b in range(4)]
        for dd in range(2):
            gru_cell(2 + dd, xs_o0, WG_IH1 + dd * 3072,
                     [hcols[:, 4 + 2 * dd:5 + 2 * dd], hcols[:, 5 + 2 * dd:6 + 2 * dd]],
                     WG_HH1 + dd * 1536,
                     frow[0:1, (2 + dd) * H:(3 + dd) * H],
                     hid_out[0:1, (2 + dd) * H:(3 + dd) * H])

        nc.sync.dma_start(d["nh_out"][:], hid_out[:])
        out1_col = row2col(hid_out[0:1, 512:1024], 4, "out1_col", False)

        # switch ACT to the exp/ln set during the big matvec
        scr3 = rows.tile([1, 2], F32, tag="scr3")
        nc.scalar.activation(scr3[:], hid_out[0:1, 767:769], AF.Ln,
                             bias=1.0, scale=0.0)

        # ---- big matvec: 13 N-tiles of 512, streamed to ag_in ----
        ag_in = dram.tile([1, VS], F32, tag="ag_in")
        for g, gw in enumerate(NG_W):
            for j in range(gw // 512):
                ps = mvp.tile([1, 512], F32, tag="mv")
                for b in range(4):
                    nc.tensor.matmul(ps[:], lhsT=out1_col[:, b:b + 1],
                                     rhs=wslab[(b, g)][:, j * 512:(j + 1) * 512],
                                     start=(b == 0), stop=(b == 3))
                lg = lgp.tile([1, 512], F32, tag="lg")
                nc.scalar.copy(lg[:], ps[:])
                n0 = goff[g] + j * 512
                nc.sync.dma_start(ag_in[0:1, n0:n0 + 512], lg[:])

        # ---- tail: local sumexp, 32B AllGather, logp shard ----
        lpm = sb.tile([128, SW], F32, tag="lpm")
        nc.sync.dma_start(lpm[:], ag_in[:].rearrange("a (p j) -> (a p) j", p=128))
        nc.vector.tensor_tensor(lpm[:], lpm[:], outb_pm, op=ALU.add)
        exp_pm = sb.tile([128, SW], F32, tag="exp_pm")
        sums = sb.tile([128, 1], F32, tag="sums")
        nc.scalar.activation(exp_pm[:], lpm[:], AF.Exp, accum_out=sums[:])
        ps_s = cp.tile([1, 1], F32, tag="cps")
        nc.tensor.matmul(ps_s[:], lhsT=ones_col, rhs=sums[:], start=True, stop=True)
        s_sb = rows.tile([1, 1], F32, tag="s_sb")
        nc.vector.tensor_copy(s_sb[:], ps_s[:])
        s_in = dram.tile([1, 1], F32, tag="s_in")
        s_out = dram.tile([1, NC], F32, tag="s_out")
        nc.sync.dma_start(s_in[:], s_sb[:])
        nc.gpsimd.collective_compute(
            "AllGather", ALU.bypass, replica_groups=[list(range(NC))],
            ins=[s_in[:].opt()], outs=[s_out[:].opt()])
        s_row = rows.tile([1, NC], F32, tag="s_row")
        nc.sync.dma_start(s_row[:], s_out[:])
        stot = rows.tile([1, 1], F32, tag="stot")
        nc.vector.reduce_sum(stot[:], s_row[:], axis=mybir.AxisListType.X)
        lse = rows.tile([1, 1], F32, tag="lse")
        nc.scalar.activation(lse[:], stot[:], AF.Ln)
        ps_b = cp.tile([128, 1], F32, tag="cps")
        nc.tensor.matmul(ps_b[:], lhsT=ones_row, rhs=lse[:], start=True, stop=True)
        lse_col = sb.tile([128, 1], F32, tag="lse_col")
        nc.vector.tensor_copy(lse_col[:], ps_b[:])
        logp_pm = sb.tile([128, SW], F32, tag="logp_pm")
        nc.vector.tensor_scalar(logp_pm[:], lpm[:], lse_col[:, 0:1], None,
                                op0=ALU.subtract)
        nc.sync.dma_start(d["logp_out"][:], logp_pm[:])


def _stage(inputs):
    import ml_dtypes
    bf16 = ml_dtypes.bfloat16
    f32 = lambda a: np.ascontiguousarray(np.asarray(a, dtype=np.float32))
    hidden = f32(inputs["hidden"])
    out_W, out_b = f32(inputs["out_W"]), f32(inputs["out_b"])
    w_ih0, w_hh0 = f32(inputs["w_ih0"]), f32(inputs["w_hh0"])
    w_ih1, w_hh1 = f32(inputs["w_ih1"]), f32(inputs["w_hh1"])
    b_ih0, b_hh0 = f32(inputs["b_ih0"]), f32(inputs["b_hh0"])
    b_ih1, b_hh1 = f32(inputs["b_ih1"]), f32(inputs["b_hh1"])

    consts2 = np.zeros((128, 184), np.float32)
    consts2[:, 0] = 1.0
    consts2[0, 4:132] = 1.0
    outb_pad = np.full(VS * NC, NEG_BIG, np.float32)
    outb_pad[:V] = out_b

    # bias_bf: attnb | combb | bih(c0..c3) | bhh(c0..c3)
    bih = [b_ih0[0], b_ih0[1], b_ih1[0], b_ih1[1]]
    bhh = [b_hh0[0], b_hh0[1], b_hh1[0], b_hh1[1]]
    bias_bf = np.concatenate(
        [f32(inputs["attn_b"]).reshape(L), f32(inputs["comb_b"]).reshape(H)]
        + bih + bhh).astype(bf16).reshape(1, -1)
    assert bias_bf.shape[1] == 768 + 8 * 768

    hcols = np.concatenate(
        [hidden[dd, 0].reshape(2, 128).T for dd in range(4)], axis=1)
    wchain = np.concatenate([
        _pm(np.ascontiguousarray(f32(inputs["attn_W"]).T)),
        _pm(f32(inputs["context_vector"])),
        _pm(np.ascontiguousarray(f32(inputs["comb_W"]).T)),
        hcols], axis=1).astype(bf16)
    assert wchain.shape == (128, 5640)
    wgru = np.concatenate(
        [_pm(np.ascontiguousarray(w_ih0[dd].T)) for dd in range(2)]
        + [_pm(np.ascontiguousarray(w_hh0[dd].T)) for dd in range(2)]
        + [_pm(np.ascontiguousarray(w_ih1[dd].T)) for dd in range(2)]
        + [_pm(np.ascontiguousarray(w_hh1[dd].T)) for dd in range(2)],
        axis=1).astype(bf16)
    assert wgru.shape == (128, 15360)

    idx = int(np.asarray(inputs["input_ids"]).reshape(-1)[0])
    com = {
        "idx2": np.full((2, 1), idx, np.int32),
        "emb": f32(inputs["emb"]),
        "frow": np.ascontiguousarray(hidden.reshape(1, 4 * H)),
        "bias_bf": np.ascontiguousarray(bias_bf),
        "wchain": np.ascontiguousarray(wchain),
        "wgru": np.ascontiguousarray(wgru),
    }
    in_maps = []
    for c in range(NC):
        lo, hi = c * VS, min(c * VS + VS, V)
        wsh = np.zeros((VS, 512), np.float32)
        if hi > lo:
            wsh[: hi - lo] = out_W[lo:hi]
        m = dict(com)
        m["outw_pm"] = _pm(np.ascontiguousarray(wsh.T)).astype(bf16)
        cc = consts2.copy()
        cc[:, 132:184] = outb_pad[lo:lo + VS].reshape(128, SW)
        m["consts2"] = cc
        in_maps.append(m)
    return in_maps


def kernel(**inputs):
    global LAST_EXEC_NS
    if "nc" not in _CACHE:
        _CACHE["nc"] = _build_nc()
    in_maps = _stage(inputs)
    res = run_bass_kernel_spmd(_CACHE["nc"], in_maps, core_ids=list(range(NC)))
    LAST_EXEC_NS = res.exec_time_ns
    _CACHE["last_results"] = res
    r0 = res.results[0]
    logp = np.concatenate(
        [res.results[c]["logp_out"].reshape(VS) for c in range(NC)])[:V]
    logp = logp.reshape(1, V).astype(np.float32)
    nh = r0["nh_out"].reshape(4, 1, H).astype(np.float32)
    aw = r0["aw_out"].reshape(1, L).astype(np.float32)
    return logp, nh, aw
